# revision 43
# baseline (speedup 1.0000x reference)
"""Mixture-of-Experts (top-2 of 8 experts, erf-GELU FFN) on 8 Trainium2
NeuronCores, expert-parallel: core e owns expert e's weights and processes
only the tokens routed to expert e.

Host side (inside kernel()): router softmax + top-2 + renormalized combine
weights, token dispatch (gather per expert) and combine (scatter-add).
Device side (Bass/Tile SPMD): per-core FFN
    y = gelu(xg @ W1[e] + b1[e]) @ W2[e] + b2[e], scaled by combine weight,
with bf16 matmuls and fp32 accumulation.

Schedule notes (evolved v1->v6 from ntff traces; exec = head + PE matmul
stream + tail, and the 282k-moving-column stream IS the roofline):
  - HAM cold-start: the PE runs at 1.2 GHz until ~3.4us of sustained
    matmul activity; ~8 dummy 512-col matmuls on a zeroed tile warm it
    up during the input-DMA head so the real stream starts at 2.4 GHz.
  - DMA queue discipline (the whole game): a DMA's completion semaphore
    fires when its LAST slice lands and a queue interleaves packets
    across outstanding DMAs, so urgent transfers must lead their FIFO;
    the ~330 GB/s HBM ceiling is split across busy queues by an opaque
    arbiter, so each FIFO is ordered by deadline, w1 (the 143 GB/s
    pacer) is alone on sync, and bulk with late deadlines (w2b) hides
    behind the w1 pool's buffer gating.
  - w2 is split into dt-halves so MM2's dt loop consumes it
    progressively, halving how much w2 must arrive by MM2's start.
  - A dummy 1-col gelu preloads the ~3us ACT table during the head.
  - bf16 outputs, one DMA per dt, keep the tail-exposed transfer small.

Layouts shipped per core (P=128 partitions, C = token capacity padded to
2*CH, CH = chunk length):
  xt0/xt1 [P, D/128, CH] bf16   xt{c}[p,db,j] = x_gathered[c*CH+j, db*128+p]
  w1_i [P, n_i, D/128, P] bf16  fb groups (1,1,2,4,4,...); tile-major:
                                [p, j, db, q] = W1[e][db*128+p, fb*128+q]
  w2a/w2b [P, F/128, D/2] bf16  w2{a,b}[p,fb,d] = W2[e][fb*128+p, d(+D/2)]
  b1  [P, F/128]        f32     b1[p,fb]   = b1[e][fb*128+p]
  cw  [P, C+D/128]      f32     [:, :C] combine weight (bcast over p),
                                [:, C+dt] = b2[e][dt*128+p]
  out [D/128, P, C]     bf16    out[dt,p,c] = y[c, dt*128+p]
"""

import numpy as np
import ml_dtypes

P = 128
N_CORES = 8

_cache = {}
_last_in_maps = None


def _build(C, D, F):
    """Build + compile the per-core SPMD Bass program for padded capacity C."""
    from concourse import bacc
    import concourse.tile as tile
    import concourse.mybir as mybir

    nb_d = D // P          # D-tiles (contraction of MM1, output tiles of MM2)
    nb_f = F // P          # F-tiles (contraction of MM2)

    n_chunks = (C + 511) // 512
    CH = C // n_chunks
    assert CH * n_chunks == C and CH % 4 == 0
    chunks = [(i * CH, CH) for i in range(n_chunks)]

    bf16 = mybir.dt.bfloat16
    f32 = mybir.dt.float32
    GELU = mybir.ActivationFunctionType.Gelu

    # w1 group structure: fb0 alone (unblocks the first matmul ASAP),
    # fb1, fb2-3, then groups of 4 -- fine-grained heads so the early
    # per-fb completion semaphores track the PE's ramp.
    groups = [(0, 1), (1, 1), (2, 2)] + [(s, 4) for s in range(4, nb_f, 4)]

    nc = bacc.Bacc(None, target_bir_lowering=False)
    xt_d = [
        nc.dram_tensor(f"xt{i}", [P, nb_d, CH], bf16, kind="ExternalInput")
        for i in range(n_chunks)
    ]
    w1_d = [
        nc.dram_tensor(f"w1_{i}", [P, n, nb_d, P], bf16, kind="ExternalInput")
        for i, (_, n) in enumerate(groups)
    ]
    hd = D // 2
    w2a_d = nc.dram_tensor("w2a", [P, nb_f, hd], bf16, kind="ExternalInput")
    w2b_d = nc.dram_tensor("w2b", [P, nb_f, hd], bf16, kind="ExternalInput")
    b1_d = nc.dram_tensor("b1", [P, nb_f], f32, kind="ExternalInput")
    cw_d = nc.dram_tensor("cw", [P, C + nb_d], f32, kind="ExternalInput")
    out_d = nc.dram_tensor("out", [nb_d, P, C], bf16, kind="ExternalOutput")

    with tile.TileContext(nc) as tc:
        with (
            tc.tile_pool(name="const", bufs=1) as const,
            tc.tile_pool(name="w1p", bufs=3) as w1p,
            tc.tile_pool(name="ps1", bufs=3, space="PSUM") as ps1p,
            tc.tile_pool(name="ps2", bufs=4, space="PSUM") as ps2p,
            tc.tile_pool(name="pwm", bufs=1, space="PSUM") as pwmp,
            tc.tile_pool(name="outp", bufs=3) as outp,
        ):
            zt = const.tile([P, 512], bf16)
            xt_t = [
                const.tile([P, nb_d, CH], bf16, name=f"xt{i}_t")
                for i in range(n_chunks)
            ]
            b1_t = const.tile([P, nb_f], f32)
            cw_t = const.tile([P, C + nb_d], f32)
            w2a_t = const.tile([P, nb_f, hd], bf16)
            w2b_t = const.tile([P, nb_f, hd], bf16)
            h_t = const.tile([P, nb_f, C], bf16)
            w1ab = [
                const.tile([P, n, nb_d, P], bf16, name=f"w1ab{i}_t")
                for i, (_, n) in enumerate(groups[:3])
            ]
            dum = const.tile([P, 1], f32)

            # ---- PE warm-up: HAM un-throttles (1.2 -> 2.4 GHz) only after
            # ~3.4us of sustained matmul activity, and re-throttles after
            # any >3.4us idle. The input-DMA path ramps slowly for the
            # first ~10us (run-dependent), so the xt/w1-head gates land
            # ~12-17us; dummy matmuls on a zeroed tile bridge the PE from
            # the preamble (~8us) to ~15us: 10x512-col to fire HAM, then
            # 32x256-col filler (fine-grained handoff). memset on Vector
            # (otherwise idle, earliest past the preamble barrier).
            nc.vector.memset(zt[:], 0.0)
            pw = pwmp.tile([P, 512], f32)
            for _ in range(10):
                nc.tensor.matmul(
                    pw[:], lhsT=zt[:, :P], rhs=zt[:], start=True, stop=True
                )
            for _ in range(12):
                nc.tensor.matmul(
                    pw[:, :256], lhsT=zt[:, :P], rhs=zt[:, :256],
                    start=True, stop=True,
                )

            # ---- input DMA: per-queue issue order is the schedule.
            # Rules learned from traces: (1) a DMA's semaphore fires when
            # its LAST slice lands, and a queue interleaves packets across
            # its outstanding DMAs -- urgent small transfers must lead
            # their queue FIFO; (2) the ~330 GB/s HBM ceiling is split
            # across busy queues by an opaque packet arbiter -- so order
            # every FIFO by deadline and keep w1 (the 143 GB/s pacer)
            # ALONE on sync, where the w1 pool's buffer gating also slots
            # w2b into sync's late-MM1 idle; (3) w2 is split by dt-halves
            # so w2b's deadline is MM2's midpoint, not its start.
            # sync:   w1(fb0), w1(fb1), w1(fb2-3), g2..g6, w2b x2, out x8
            # scalar: xt0 lo/hi, xt1 lo, gelu-table preload, w2a (in loop)
            # gpsimd: xt1 hi, b1, cw, g0, g1
            # g0/g1 ride gpsimd's tail so the slow-ramping early window
            # moves only true gates; sync's g3+ are pool-gated anyway.
            # (pool writers chain in allocation order, so group DMAs are
            # emitted strictly in g0..g6 order; per-queue FIFO position
            # comes from emitting each engine's lead-ins first)
            for i in range(3):
                nc.sync.dma_start(w1ab[i][:], w1_d[i][:])

            # xt in db-half pieces (the first accumulation group's gate is
            # the smallest possible transfer); chunk 1's upper half rides
            # the gpsimd queue. Only gpsimd/sync/scalar can issue DMAs.
            hb = nb_d // 2
            nc.scalar.dma_start(xt_t[0][:, :hb], xt_d[0][:, :hb])
            nc.scalar.dma_start(xt_t[0][:, hb:], xt_d[0][:, hb:])
            for i in range(1, n_chunks):
                nc.scalar.dma_start(xt_t[i][:, :hb], xt_d[i][:, :hb])
                nc.gpsimd.dma_start(xt_t[i][:, hb:], xt_d[i][:, hb:])
            nc.scalar.activation(dum[:], zt[:, :1], GELU)

            nc.gpsimd.dma_start(b1_t[:], b1_d[:])
            nc.gpsimd.dma_start(cw_t[:], cw_d[:])

            w1g_t = []
            for i in range(3, len(groups)):
                g = w1p.tile([P, 4, nb_d, P], bf16, name="w1g_t")
                eng = nc.gpsimd if i < 5 else nc.sync
                eng.dma_start(g[:], w1_d[i][:])
                w1g_t.append(g)

            hf = nb_f // 2
            nc.sync.dma_start(w2b_t[:, :hf, :], w2b_d[:, :hf, :])
            nc.sync.dma_start(w2b_t[:, hf:, :], w2b_d[:, hf:, :])

            # ---- MM1: hT[fb] = gelu(sum_db w1[db,fb]^T @ xT[db] + b1[fb])
            def w1_tile(fb):
                if fb < 2:
                    return w1ab[fb][:, 0]
                if fb < 4:
                    return w1ab[2][:, fb - 2]
                return w1g_t[(fb - 4) // 4][:, fb % 4]

            for fb in range(nb_f):
                wt = w1_tile(fb)
                for ci, (c0, cn) in enumerate(chunks):
                    ps = ps1p.tile([P, 512], f32)
                    for db in range(nb_d):
                        nc.tensor.matmul(
                            ps[:, :cn],
                            lhsT=wt[:, db, :],
                            rhs=xt_t[ci][:, db, :],
                            start=(db == 0),
                            stop=(db == nb_d - 1),
                        )
                    nc.scalar.activation(
                        h_t[:, fb, c0 : c0 + cn],
                        ps[:, :cn],
                        GELU,
                        bias=b1_t[:, fb : fb + 1],
                    )
                # w2a issues ride the scalar engine between early MM1
                # evictions, clearing the head's DMA-crunch window.
                if 3 <= fb <= 6:
                    q = fb - 3
                    nc.scalar.dma_start(
                        w2a_t[:, q * 8 : (q + 1) * 8, :],
                        w2a_d[:, q * 8 : (q + 1) * 8, :],
                    )

            # ---- MM2: yT[dt] = (sum_fb w2[fb,dt]^T @ hT[fb]) + b2, * wg
            # bf16 output, DMA'd per dt (keeps the final, tail-exposed
            # transfer small).
            for dt in range(nb_d):
                o_t = outp.tile([P, C], bf16)
                w2h = w2a_t if dt < nb_d // 2 else w2b_t
                dh = dt % (nb_d // 2)
                for c0, cn in chunks:
                    ps = ps2p.tile([P, 512], f32)
                    for fb in range(nb_f):
                        nc.tensor.matmul(
                            ps[:, :cn],
                            lhsT=w2h[:, fb, dh * P : (dh + 1) * P],
                            rhs=h_t[:, fb, c0 : c0 + cn],
                            start=(fb == 0),
                            stop=(fb == nb_f - 1),
                        )
                    nc.vector.scalar_tensor_tensor(
                        o_t[:, c0 : c0 + cn],
                        ps[:, :cn],
                        cw_t[:, C + dt : C + dt + 1],
                        cw_t[:, c0 : c0 + cn],
                        op0=mybir.AluOpType.add,
                        op1=mybir.AluOpType.mult,
                    )
                nc.sync.dma_start(out_d[dt], o_t[:])

    nc.compile()
    return nc


def _route(x, W_router):
    """Top-2 routing, replicating jax softmax/top_k/renorm semantics."""
    T = x.shape[0]
    logits = x @ np.asarray(W_router, np.float32)
    m = logits.max(axis=1, keepdims=True)
    ex = np.exp(logits - m, dtype=np.float32)
    probs = ex / ex.sum(axis=1, keepdims=True, dtype=np.float32)
    r = np.arange(T)
    i1 = probs.argmax(axis=1)
    masked = probs.copy()
    masked[r, i1] = -np.inf
    i2 = masked.argmax(axis=1)
    p1 = probs[r, i1]
    p2 = probs[r, i2]
    s = p1 + p2
    return i1, i2, p1 / s, p2 / s


def kernel(hidden_states, W_router, W1, b1, W2, b2):
    from concourse.bass_utils import run_bass_kernel_spmd

    B, S, D = hidden_states.shape
    E, _, F = W1.shape
    T = B * S
    x = np.ascontiguousarray(np.asarray(hidden_states, np.float32).reshape(T, D))

    i1, i2, w1c, w2c = _route(x, W_router)

    idxs, wgts = [], []
    for e in range(E):
        sel1 = i1 == e
        sel2 = i2 == e
        idx = np.nonzero(sel1 | sel2)[0]
        w = np.where(sel1[idx], w1c[idx], w2c[idx]).astype(np.float32)
        idxs.append(idx)
        wgts.append(w)

    Craw = max(max(len(ix) for ix in idxs), 1)
    nb_d = D // P
    nb_f = F // P
    n_chunks = (Craw + 511) // 512
    CH = -(-Craw // (n_chunks * 4)) * 4     # chunk len, multiple of 4
    C = CH * n_chunks                        # padded capacity

    key = (C, D, F)
    if key not in _cache:
        _cache[key] = _build(C, D, F)
    nc = _cache[key]

    bf16 = ml_dtypes.bfloat16
    W1b = np.asarray(W1, np.float32).astype(bf16)
    W2b = np.asarray(W2, np.float32).astype(bf16)
    xb = x.astype(bf16)

    groups = [(0, 1), (1, 1), (2, 2)] + [(s, 4) for s in range(4, nb_f, 4)]

    in_maps = []
    for e in range(E):
        n = len(idxs[e])
        xg = np.zeros((C, D), bf16)
        xg[:n] = xb[idxs[e]]
        m = {}
        for i in range(n_chunks):
            xc = xg[i * CH : (i + 1) * CH]  # [CH, D]
            m[f"xt{i}"] = np.ascontiguousarray(
                xc.T.reshape(nb_d, P, CH).transpose(1, 0, 2)
            )
        # w1 tile-major, grouped: [p, fb-in-group, db, q]
        w1all = W1b[e].reshape(nb_d, P, nb_f, P).transpose(2, 1, 0, 3)
        for i, (s, ng) in enumerate(groups):
            m[f"w1_{i}"] = np.ascontiguousarray(
                w1all[s : s + ng].transpose(1, 0, 2, 3)
            )
        w2e = W2b[e].reshape(nb_f, P, D).transpose(1, 0, 2)
        m["w2a"] = np.ascontiguousarray(w2e[:, :, : D // 2])
        m["w2b"] = np.ascontiguousarray(w2e[:, :, D // 2 :])
        m["b1"] = np.ascontiguousarray(np.asarray(b1[e], np.float32).reshape(nb_f, P).T)
        cw = np.zeros((P, C + nb_d), np.float32)
        cw[:, :n] = wgts[e]
        cw[:, C:] = np.asarray(b2[e], np.float32).reshape(nb_d, P).T
        m["cw"] = cw
        in_maps.append(m)

    global _last_in_maps
    _last_in_maps = in_maps

    res = run_bass_kernel_spmd(nc, in_maps, core_ids=list(range(N_CORES)))

    out = np.zeros((T, D), np.float32)
    for e in range(E):
        n = len(idxs[e])
        # device out[dt, p, c] = y[c, dt*128+p]
        y = (
            np.asarray(res.results[e]["out"])
            .astype(np.float32)
            .transpose(2, 0, 1)
            .reshape(C, D)[:n]
        )
        out[idxs[e]] += y
    return out.reshape(B, S, D).astype(np.float32)


# revision 45
# speedup vs baseline: 1.2818x; 1.2818x over previous
"""Mixture-of-Experts (top-2 of 8 experts, erf-GELU FFN) on 8 Trainium2
NeuronCores, expert-parallel: core e owns expert e's weights and processes
only the tokens routed to expert e.

Host side (inside kernel()): router softmax + top-2 + renormalized combine
weights, token dispatch (gather per expert) and combine (scatter-add).
Device side (Bass/Tile SPMD): per-core FFN
    y = gelu(xg @ W1[e] + b1[e]) @ W2[e] + b2[e], scaled by combine weight,
with bf16 matmuls and fp32 accumulation.

Schedule notes (evolved v1->v6 from ntff traces; exec = head + PE matmul
stream + tail, and the 282k-moving-column stream IS the roofline):
  - HAM cold-start: the PE runs at 1.2 GHz until ~3.4us of sustained
    matmul activity; ~8 dummy 512-col matmuls on a zeroed tile warm it
    up during the input-DMA head so the real stream starts at 2.4 GHz.
  - DMA queue discipline (the whole game): a DMA's completion semaphore
    fires when its LAST slice lands and a queue interleaves packets
    across outstanding DMAs, so urgent transfers must lead their FIFO;
    the ~330 GB/s HBM ceiling is split across busy queues by an opaque
    arbiter, so each FIFO is ordered by deadline, w1 (the 143 GB/s
    pacer) is alone on sync, and bulk with late deadlines (w2b) hides
    behind the w1 pool's buffer gating.
  - w2 is split into dt-halves so MM2's dt loop consumes it
    progressively, halving how much w2 must arrive by MM2's start.
  - A dummy 1-col gelu preloads the ~3us ACT table during the head.
  - bf16 outputs, one DMA per dt, keep the tail-exposed transfer small.

Layouts shipped per core (P=128 partitions, C = token capacity padded to
2*CH, CH = chunk length):
  xt0/xt1 [P, D/128, CH] bf16   xt{c}[p,db,j] = x_gathered[c*CH+j, db*128+p]
  w1_i [P, n_i, D/128, P] bf16  fb groups (1,1,2,4,4,...); tile-major:
                                [p, j, db, q] = W1[e][db*128+p, fb*128+q]
  w2a/w2b [P, F/128, D/2] bf16  w2{a,b}[p,fb,d] = W2[e][fb*128+p, d(+D/2)]
  b1  [P, F/128]        f32     b1[p,fb]   = b1[e][fb*128+p]
  cw  [P, C+D/128]      f32     [:, :C] combine weight (bcast over p),
                                [:, C+dt] = b2[e][dt*128+p]
  out [D/128, P, C]     bf16    out[dt,p,c] = y[c, dt*128+p]
"""

import numpy as np
import ml_dtypes

P = 128
N_CORES = 8

_cache = {}
_last_in_maps = None


def _build(C, D, F):
    """Build + compile the per-core SPMD Bass program for padded capacity C."""
    from concourse import bacc
    import concourse.tile as tile
    import concourse.mybir as mybir

    nb_d = D // P          # D-tiles (contraction of MM1, output tiles of MM2)
    nb_f = F // P          # F-tiles (contraction of MM2)

    n_chunks = (C + 511) // 512
    CH = C // n_chunks
    assert CH * n_chunks == C and CH % 4 == 0
    chunks = [(i * CH, CH) for i in range(n_chunks)]

    bf16 = mybir.dt.bfloat16
    f32 = mybir.dt.float32
    GELU = mybir.ActivationFunctionType.Gelu

    # w1 group structure: fb0 alone (unblocks the first matmul ASAP),
    # fb1, fb2-3, then groups of 4 -- fine-grained heads so the early
    # per-fb completion semaphores track the PE's ramp.
    groups = [(0, 1), (1, 1), (2, 2)] + [(s, 4) for s in range(4, nb_f, 4)]

    nc = bacc.Bacc(None, target_bir_lowering=False)
    xt_d = [
        nc.dram_tensor(f"xt{i}", [P, nb_d, CH], bf16, kind="ExternalInput")
        for i in range(n_chunks)
    ]
    w1_d = [
        nc.dram_tensor(f"w1_{i}", [P, n, nb_d, P], bf16, kind="ExternalInput")
        for i, (_, n) in enumerate(groups)
    ]
    hd = D // 2
    w2a_d = nc.dram_tensor("w2a", [P, nb_f, hd], bf16, kind="ExternalInput")
    w2b_d = nc.dram_tensor("w2b", [P, nb_f, hd], bf16, kind="ExternalInput")
    b1_d = nc.dram_tensor("b1", [P, nb_f], f32, kind="ExternalInput")
    cw_d = nc.dram_tensor("cw", [P, C + nb_d], f32, kind="ExternalInput")
    out_d = nc.dram_tensor("out", [nb_d, P, C], bf16, kind="ExternalOutput")

    with tile.TileContext(nc) as tc:
        with (
            tc.tile_pool(name="const", bufs=1) as const,
            tc.tile_pool(name="w1p", bufs=3) as w1p,
            tc.tile_pool(name="ps1", bufs=3, space="PSUM") as ps1p,
            tc.tile_pool(name="ps2", bufs=4, space="PSUM") as ps2p,
            tc.tile_pool(name="pwm", bufs=1, space="PSUM") as pwmp,
            tc.tile_pool(name="outp", bufs=3) as outp,
        ):
            zt = const.tile([P, 512], bf16)
            xt_t = [
                const.tile([P, nb_d, CH], bf16, name=f"xt{i}_t")
                for i in range(n_chunks)
            ]
            b1_t = const.tile([P, nb_f], f32)
            cw_t = const.tile([P, C + nb_d], f32)
            w2a_t = const.tile([P, nb_f, hd], bf16)
            w2b_t = const.tile([P, nb_f, hd], bf16)
            h_t = const.tile([P, nb_f, C], bf16)
            w1ab = [
                const.tile([P, n, nb_d, P], bf16, name=f"w1ab{i}_t")
                for i, (_, n) in enumerate(groups[:3])
            ]
            dum = const.tile([P, 1], f32)

            # ---- PE warm-up: HAM un-throttles (1.2 -> 2.4 GHz) only after
            # ~3.4us of sustained matmul activity, and re-throttles after
            # any >3.4us idle. The input-DMA path ramps slowly for the
            # first ~10us (run-dependent), so the xt/w1-head gates land
            # ~12-17us; dummy matmuls on a zeroed tile bridge the PE from
            # the preamble (~8us) to ~15us: 10x512-col to fire HAM, then
            # 32x256-col filler (fine-grained handoff). memset on Vector
            # (otherwise idle, earliest past the preamble barrier).
            nc.vector.memset(zt[:], 0.0)
            pw = pwmp.tile([P, 512], f32)
            for _ in range(10):
                nc.tensor.matmul(
                    pw[:], lhsT=zt[:, :P], rhs=zt[:], start=True, stop=True
                )
            for _ in range(12):
                nc.tensor.matmul(
                    pw[:, :256], lhsT=zt[:, :P], rhs=zt[:, :256],
                    start=True, stop=True,
                )

            # ---- input DMA: per-queue issue order is the schedule.
            # Rules learned from traces: (1) a DMA's semaphore fires when
            # its LAST slice lands, and a queue interleaves packets across
            # its outstanding DMAs -- urgent small transfers must lead
            # their queue FIFO; (2) the ~330 GB/s HBM ceiling is split
            # across busy queues by an opaque packet arbiter -- so order
            # every FIFO by deadline and keep w1 (the 143 GB/s pacer)
            # ALONE on sync, where the w1 pool's buffer gating also slots
            # w2b into sync's late-MM1 idle; (3) w2 is split by dt-halves
            # so w2b's deadline is MM2's midpoint, not its start.
            # sync:   w1(fb0), w1(fb1), w1(fb2-3), g0..g6, w2b x2, out x8
            # scalar: xt0 lo/hi, xt1 lo, gelu-table preload, w2a (in loop)
            # gpsimd: xt1 hi, b1, cw   (SWDGE ramps slowest -- keep light)
            for i in range(3):
                nc.sync.dma_start(w1ab[i][:], w1_d[i][:])

            # xt in db-half pieces (the first accumulation group's gate is
            # the smallest possible transfer); chunk 1's upper half rides
            # the gpsimd queue. Only gpsimd/sync/scalar can issue DMAs.
            hb = nb_d // 2
            nc.scalar.dma_start(xt_t[0][:, :hb], xt_d[0][:, :hb])
            nc.scalar.dma_start(xt_t[0][:, hb:], xt_d[0][:, hb:])
            for i in range(1, n_chunks):
                nc.scalar.dma_start(xt_t[i][:, :hb], xt_d[i][:, :hb])
                nc.gpsimd.dma_start(xt_t[i][:, hb:], xt_d[i][:, hb:])
            nc.scalar.activation(dum[:], zt[:, :1], GELU)

            nc.gpsimd.dma_start(b1_t[:], b1_d[:])
            nc.gpsimd.dma_start(cw_t[:], cw_d[:])

            w1g_t = []
            for i in range(3, len(groups)):
                g = w1p.tile([P, 4, nb_d, P], bf16, name="w1g_t")
                nc.sync.dma_start(g[:], w1_d[i][:])
                w1g_t.append(g)

            hf = nb_f // 2
            nc.sync.dma_start(w2b_t[:, :hf, :], w2b_d[:, :hf, :])
            nc.sync.dma_start(w2b_t[:, hf:, :], w2b_d[:, hf:, :])

            # ---- MM1: hT[fb] = gelu(sum_db w1[db,fb]^T @ xT[db] + b1[fb])
            def w1_tile(fb):
                if fb < 2:
                    return w1ab[fb][:, 0]
                if fb < 4:
                    return w1ab[2][:, fb - 2]
                return w1g_t[(fb - 4) // 4][:, fb % 4]

            for fb in range(nb_f):
                wt = w1_tile(fb)
                for ci, (c0, cn) in enumerate(chunks):
                    ps = ps1p.tile([P, 512], f32)
                    for db in range(nb_d):
                        nc.tensor.matmul(
                            ps[:, :cn],
                            lhsT=wt[:, db, :],
                            rhs=xt_t[ci][:, db, :],
                            start=(db == 0),
                            stop=(db == nb_d - 1),
                        )
                    nc.scalar.activation(
                        h_t[:, fb, c0 : c0 + cn],
                        ps[:, :cn],
                        GELU,
                        bias=b1_t[:, fb : fb + 1],
                    )
                # w2a issues ride the scalar engine between early MM1
                # evictions, clearing the head's DMA-crunch window.
                if 3 <= fb <= 6:
                    q = fb - 3
                    nc.scalar.dma_start(
                        w2a_t[:, q * 8 : (q + 1) * 8, :],
                        w2a_d[:, q * 8 : (q + 1) * 8, :],
                    )

            # ---- MM2: yT[dt] = (sum_fb w2[fb,dt]^T @ hT[fb]) + b2, * wg
            # bf16 output, DMA'd per dt (keeps the final, tail-exposed
            # transfer small).
            for dt in range(nb_d):
                o_t = outp.tile([P, C], bf16)
                w2h = w2a_t if dt < nb_d // 2 else w2b_t
                dh = dt % (nb_d // 2)
                for c0, cn in chunks:
                    ps = ps2p.tile([P, 512], f32)
                    for fb in range(nb_f):
                        nc.tensor.matmul(
                            ps[:, :cn],
                            lhsT=w2h[:, fb, dh * P : (dh + 1) * P],
                            rhs=h_t[:, fb, c0 : c0 + cn],
                            start=(fb == 0),
                            stop=(fb == nb_f - 1),
                        )
                    nc.vector.scalar_tensor_tensor(
                        o_t[:, c0 : c0 + cn],
                        ps[:, :cn],
                        cw_t[:, C + dt : C + dt + 1],
                        cw_t[:, c0 : c0 + cn],
                        op0=mybir.AluOpType.add,
                        op1=mybir.AluOpType.mult,
                    )
                nc.sync.dma_start(out_d[dt], o_t[:])

    nc.compile()
    return nc


def _route(x, W_router):
    """Top-2 routing, replicating jax softmax/top_k/renorm semantics."""
    T = x.shape[0]
    logits = x @ np.asarray(W_router, np.float32)
    m = logits.max(axis=1, keepdims=True)
    ex = np.exp(logits - m, dtype=np.float32)
    probs = ex / ex.sum(axis=1, keepdims=True, dtype=np.float32)
    r = np.arange(T)
    i1 = probs.argmax(axis=1)
    masked = probs.copy()
    masked[r, i1] = -np.inf
    i2 = masked.argmax(axis=1)
    p1 = probs[r, i1]
    p2 = probs[r, i2]
    s = p1 + p2
    return i1, i2, p1 / s, p2 / s


def kernel(hidden_states, W_router, W1, b1, W2, b2):
    from concourse.bass_utils import run_bass_kernel_spmd

    B, S, D = hidden_states.shape
    E, _, F = W1.shape
    T = B * S
    x = np.ascontiguousarray(np.asarray(hidden_states, np.float32).reshape(T, D))

    i1, i2, w1c, w2c = _route(x, W_router)

    idxs, wgts = [], []
    for e in range(E):
        sel1 = i1 == e
        sel2 = i2 == e
        idx = np.nonzero(sel1 | sel2)[0]
        w = np.where(sel1[idx], w1c[idx], w2c[idx]).astype(np.float32)
        idxs.append(idx)
        wgts.append(w)

    Craw = max(max(len(ix) for ix in idxs), 1)
    nb_d = D // P
    nb_f = F // P
    n_chunks = (Craw + 511) // 512
    CH = -(-Craw // (n_chunks * 4)) * 4     # chunk len, multiple of 4
    C = CH * n_chunks                        # padded capacity

    key = (C, D, F)
    if key not in _cache:
        _cache[key] = _build(C, D, F)
    nc = _cache[key]

    bf16 = ml_dtypes.bfloat16
    W1b = np.asarray(W1, np.float32).astype(bf16)
    W2b = np.asarray(W2, np.float32).astype(bf16)
    xb = x.astype(bf16)

    groups = [(0, 1), (1, 1), (2, 2)] + [(s, 4) for s in range(4, nb_f, 4)]

    in_maps = []
    for e in range(E):
        n = len(idxs[e])
        xg = np.zeros((C, D), bf16)
        xg[:n] = xb[idxs[e]]
        m = {}
        for i in range(n_chunks):
            xc = xg[i * CH : (i + 1) * CH]  # [CH, D]
            m[f"xt{i}"] = np.ascontiguousarray(
                xc.T.reshape(nb_d, P, CH).transpose(1, 0, 2)
            )
        # w1 tile-major, grouped: [p, fb-in-group, db, q]
        w1all = W1b[e].reshape(nb_d, P, nb_f, P).transpose(2, 1, 0, 3)
        for i, (s, ng) in enumerate(groups):
            m[f"w1_{i}"] = np.ascontiguousarray(
                w1all[s : s + ng].transpose(1, 0, 2, 3)
            )
        w2e = W2b[e].reshape(nb_f, P, D).transpose(1, 0, 2)
        m["w2a"] = np.ascontiguousarray(w2e[:, :, : D // 2])
        m["w2b"] = np.ascontiguousarray(w2e[:, :, D // 2 :])
        m["b1"] = np.ascontiguousarray(np.asarray(b1[e], np.float32).reshape(nb_f, P).T)
        cw = np.zeros((P, C + nb_d), np.float32)
        cw[:, :n] = wgts[e]
        cw[:, C:] = np.asarray(b2[e], np.float32).reshape(nb_d, P).T
        m["cw"] = cw
        in_maps.append(m)

    global _last_in_maps
    _last_in_maps = in_maps

    res = run_bass_kernel_spmd(nc, in_maps, core_ids=list(range(N_CORES)))

    out = np.zeros((T, D), np.float32)
    for e in range(E):
        n = len(idxs[e])
        # device out[dt, p, c] = y[c, dt*128+p]
        y = (
            np.asarray(res.results[e]["out"])
            .astype(np.float32)
            .transpose(2, 0, 1)
            .reshape(C, D)[:n]
        )
        out[idxs[e]] += y
    return out.reshape(B, S, D).astype(np.float32)
